# revision 5
# baseline (speedup 1.0000x reference)
"""CrossAttn + SparseNormer TRN2 kernel, tensor-parallel over heads on 8 cores.

Sharding: core c computes heads {2c, 2c+1} end-to-end (Wq/Wkv sharded on
output dim, Wo on input dim); each core emits a partial (B*Q, D) output of
the final projection and the host sums the 8 partials (the "all-reduce").

Per-core dataflow (all layouts chosen so no on-device transposes are needed):
  rqT[c, bq] = (Wq_c/sqrt(ADIM)) @ iQ.T      (fp32 in SBUF, f32r matmuls)
  rkT[c, bs] = Wk_c @ iK.T
  rvo[s, (b,sc), 0:64|64] = iK @ Wv_c.T with a constant 1.0 column appended
  scoresT[s, q] = rkT.T @ rqT per head       (K=64, row-packed 2 heads)
  t = relu(scoresT + nbias)^2 * keep  ==  (r*keep)*r  with r = relu(s+b)
  po[0:64|64, q] = [rv | 1].T @ t            (attn-value + rowsum in one MM)
  oT = po[0:64] / (po[64] + eps)             (recip + PE-broadcast + mult)
  out_partial = oT.T @ Wo_c.T                (K=64 x2 heads accumulated)
"""

import os
import numpy as np
import ml_dtypes
from contextlib import ExitStack

import concourse.bass as bass
import concourse.tile as tile
from concourse import bacc, mybir
from concourse.bass import ts, ds
from concourse.bass_utils import run_bass_kernel_spmd

AF = mybir.ActivationFunctionType
ALU = mybir.AluOpType
F32 = mybir.dt.float32
F32R = mybir.dt.float32r
BF16 = mybir.dt.bfloat16

B, Q, S, D, H = 2, 2048, 2048, 1024, 16
ADIM = 64
NCORES = 8
P = 128
QB = 512          # q-block (psum free dim)
SCH = 128         # s-chunk (scores partition dim)
IEPS = 1e-32

GPS_EVERY = 2     # route every GPS_EVERY-th square-mult to gpsimd (0=off)

_last_results = None


def _body(ctx, tc, aps, Bv, Qv, Sv, nbias_val):
    nc = tc.nc
    qT, kT, keepT, wqT, wkT, wvT, woT, bpat, out = aps
    BQ, BS = Bv * Qv, Bv * Sv
    KC = D // P                      # contraction chunks for projections
    nsc_b = Sv // SCH                # s-chunks per batch
    nqt_b = Qv // QB                 # q-blocks per batch
    nsb_tot = BS // SCH              # total s-chunks

    qT_r = qT.rearrange("(o p) n -> p o n", p=P)
    kT_r = kT.rearrange("(o p) n -> p o n", p=P)

    const = ctx.enter_context(tc.tile_pool(name="const", bufs=1))
    wq_sb = const.tile([P, KC, P], BF16)
    nc.sync.dma_start(wq_sb, wqT.rearrange("(o p) m -> p o m", p=P))
    wk_sb = const.tile([P, KC, P], BF16)
    nc.sync.dma_start(wk_sb, wkT.rearrange("(o p) m -> p o m", p=P))
    wv_sb = const.tile([P, KC, P], BF16)
    nc.sync.dma_start(wv_sb, wvT.rearrange("(o p) m -> p o m", p=P))
    # Wo split per head so lhsT/rhs partition bases line up at 0
    woA_sb = const.tile([ADIM, D], BF16)
    nc.sync.dma_start(woA_sb, woT[0:ADIM, :])
    woB_sb = const.tile([ADIM, D], BF16)
    nc.sync.dma_start(woB_sb, woT[ADIM : 2 * ADIM, :])
    # broadcast pattern (host const): cols 0:64 = [[1],[0]], 64:128 = [[0],[1]]
    bpat_sb = const.tile([2, P], BF16)
    nc.sync.dma_start(bpat_sb, bpat)
    bpatA = bpat_sb[:, 0:ADIM]
    bpatB = bpat_sb[:, ADIM : 2 * ADIM]

    rqT = const.tile([P, BQ], F32R)
    rkT = const.tile([P, BS], F32R)
    rvoA = const.tile([P, nsb_tot, ADIM + 1], BF16)
    nc.any.memset(rvoA[:, :, ADIM], 1.0)
    rvoB = const.tile([P, nsb_tot, ADIM + 1], BF16)
    nc.any.memset(rvoB[:, :, ADIM], 1.0)

    io = ctx.enter_context(tc.tile_pool(name="io", bufs=3))

    # ---------------- phase 1: projections ----------------
    with tc.tile_pool(name="pp", bufs=2, space="PSUM") as pp:
        for t in range(BQ // QB):
            qld = io.tile([P, KC, QB], BF16, tag="qload")
            nc.sync.dma_start(qld, qT_r[:, :, ts(t, QB)])
            ps = pp.tile([P, QB], F32, tag="pp")
            for kc in range(KC):
                nc.tensor.matmul(ps, wq_sb[:, kc, :], qld[:, kc, :],
                                 start=(kc == 0), stop=(kc == KC - 1))
            nc.scalar.copy(rqT[:, ts(t, QB)], ps)

        for t in range(BS // QB):
            kld = io.tile([P, KC, QB], BF16, tag="qload")
            nc.sync.dma_start(kld, kT_r[:, :, ts(t, QB)])
            ps = pp.tile([P, QB], F32, tag="pp")
            for kc in range(KC):
                nc.tensor.matmul(ps, wk_sb[:, kc, :], kld[:, kc, :],
                                 start=(kc == 0), stop=(kc == KC - 1))
            nc.scalar.copy(rkT[:, ts(t, QB)], ps)
            for j in range(QB // SCH):
                sidx = t * (QB // SCH) + j
                prv = pp.tile([P, P], F32, tag="prv")
                for kc in range(KC):
                    nc.tensor.matmul(prv, kld[:, kc, ds(j * SCH, SCH)],
                                     wv_sb[:, kc, :],
                                     start=(kc == 0), stop=(kc == KC - 1))
                nc.vector.tensor_copy(rvoA[:, sidx, 0:ADIM], prv[:, 0:ADIM])
                nc.vector.tensor_copy(rvoB[:, sidx, 0:ADIM],
                                      prv[:, ADIM : 2 * ADIM])

    # ---------------- phase 2: attention ----------------
    sp = ctx.enter_context(tc.tile_pool(name="sp", bufs=3, space="PSUM"))
    pop = ctx.enter_context(tc.tile_pool(name="pop", bufs=1, space="PSUM"))
    bcp = ctx.enter_context(tc.tile_pool(name="bcp", bufs=1, space="PSUM"))
    op = ctx.enter_context(tc.tile_pool(name="op", bufs=1, space="PSUM"))
    sb2 = ctx.enter_context(tc.tile_pool(name="sb2", bufs=4))
    sb3 = ctx.enter_context(tc.tile_pool(name="sb3", bufs=2))
    kp = ctx.enter_context(tc.tile_pool(name="kp", bufs=3))

    gps_ctr = 0
    for b in range(Bv):
        for qt in range(nqt_b):
            qs = b * Qv + qt * QB
            poA = pop.tile([ADIM + 1, QB], F32, tag="poA")
            poB = pop.tile([ADIM + 1, QB], F32, tag="poB")
            for sc in range(nsc_b):
                ss = b * Sv + sc * SCH
                sidx = b * nsc_b + sc
                keep_t = kp.tile([SCH, QB], BF16, tag="keep")
                nc.sync.dma_start(
                    keep_t, keepT[b, ds(sc * SCH, SCH), ds(qt * QB, QB)])
                for half, po, rvo in ((0, poA, rvoA), (1, poB, rvoB)):
                    hs = half * ADIM
                    pscore = sp.tile([SCH, QB], F32, tag="ps_sc")
                    nc.tensor.matmul(
                        pscore,
                        rkT[hs : hs + ADIM, ds(ss, SCH)],
                        rqT[hs : hs + ADIM, ds(qs, QB)],
                        start=True, stop=True, tile_position=(hs, 0))
                    r = sb2.tile([SCH, QB], BF16, tag="r")
                    nc.scalar.activation(r, pscore, AF.Relu,
                                         bias=float(nbias_val), scale=1.0)
                    rk_ = sb2.tile([SCH, QB], BF16, tag="rk")
                    nc.vector.tensor_tensor(rk_, r, keep_t, op=ALU.mult)
                    tt = sb2.tile([SCH, QB], BF16, tag="tt")
                    gps_ctr += 1
                    if GPS_EVERY and gps_ctr % GPS_EVERY == 0:
                        nc.gpsimd.tensor_tensor(tt, rk_, r, op=ALU.mult)
                    else:
                        nc.vector.tensor_tensor(tt, rk_, r, op=ALU.mult)
                    nc.tensor.matmul(po, rvo[:, sidx, :], tt,
                                     start=(sc == 0), stop=(sc == nsc_b - 1))

            # normalization
            poA_sb = sb3.tile([ADIM + 1, QB], F32, tag="poA_sb")
            nc.scalar.copy(poA_sb, poA)
            poB_sb = sb3.tile([ADIM + 1, QB], F32, tag="poB_sb")
            nc.scalar.copy(poB_sb, poB)
            rs2 = sb3.tile([2, QB], F32, tag="rs2")
            nc.sync.dma_start(rs2[0:1, :], poA_sb[ADIM : ADIM + 1, :])
            nc.sync.dma_start(rs2[1:2, :], poB_sb[ADIM : ADIM + 1, :])
            rcp = sb3.tile([2, QB], F32, tag="rcp")
            nc.vector.tensor_scalar_add(rcp, rs2, IEPS)
            nc.vector.reciprocal(rcp, rcp)
            rcpb = sb3.tile([2, QB], BF16, tag="rcpb")
            nc.vector.tensor_copy(rcpb, rcp)
            bcA = bcp.tile([ADIM, QB], F32, tag="bcA")
            nc.tensor.matmul(bcA, bpatA, rcpb, start=True, stop=True)
            bcB = bcp.tile([ADIM, QB], F32, tag="bcB")
            nc.tensor.matmul(bcB, bpatB, rcpb, start=True, stop=True)
            oTA = sb3.tile([ADIM, QB], BF16, tag="oTA")
            nc.vector.tensor_tensor(oTA, poA_sb[0:ADIM, :], bcA, op=ALU.mult)
            oTB = sb3.tile([ADIM, QB], BF16, tag="oTB")
            nc.vector.tensor_tensor(oTB, poB_sb[0:ADIM, :], bcB, op=ALU.mult)

            # output projection (partial over this core's 128 f-dims)
            for qc in range(QB // P):
                for ec in range(D // QB):
                    pso = op.tile([P, QB], F32, tag="pso")
                    nc.tensor.matmul(pso, oTA[:, ds(qc * P, P)],
                                     woA_sb[:, ds(ec * QB, QB)],
                                     start=True, stop=False)
                    nc.tensor.matmul(pso, oTB[:, ds(qc * P, P)],
                                     woB_sb[:, ds(ec * QB, QB)],
                                     start=False, stop=True)
                    osb = sb2.tile([P, QB], F32, tag="osb")
                    nc.any.tensor_copy(osb, pso)
                    nc.sync.dma_start(
                        out[ds(qs + qc * P, P), ds(ec * QB, QB)], osb)


_nc_cache = {}


def _build(Bv, Qv, Sv, nbias_val, num_devices=NCORES):
    key = (Bv, Qv, Sv, float(nbias_val), num_devices)
    if key in _nc_cache:
        return _nc_cache[key]
    nc = bacc.Bacc("TRN2", target_bir_lowering=False, debug=False,
                   num_devices=num_devices)
    BQ, BS = Bv * Qv, Bv * Sv
    qT = nc.dram_tensor("qT", [D, BQ], BF16, kind="ExternalInput").ap()
    kT = nc.dram_tensor("kT", [D, BS], BF16, kind="ExternalInput").ap()
    keepT = nc.dram_tensor("keepT", [Bv, Sv, Qv], BF16,
                           kind="ExternalInput").ap()
    wqT = nc.dram_tensor("wqT", [D, P], BF16, kind="ExternalInput").ap()
    wkT = nc.dram_tensor("wkT", [D, P], BF16, kind="ExternalInput").ap()
    wvT = nc.dram_tensor("wvT", [D, P], BF16, kind="ExternalInput").ap()
    woT = nc.dram_tensor("woT", [P, D], BF16, kind="ExternalInput").ap()
    bpat = nc.dram_tensor("bpat", [2, P], BF16, kind="ExternalInput").ap()
    out = nc.dram_tensor("out", [BQ, D], F32, kind="ExternalOutput").ap()
    aps = (qT, kT, keepT, wqT, wkT, wvT, woT, bpat, out)
    with tile.TileContext(nc) as tc:
        with ExitStack() as ctx:
            _body(ctx, tc, aps, Bv, Qv, Sv, nbias_val)
    nc.compile()
    _nc_cache[key] = nc
    return nc


def _prep_inputs(iQ, iK, mask, Wq, Wkv, Wo, nbias):
    Bv, Qv, _ = iQ.shape
    Sv = iK.shape[1]
    bf = ml_dtypes.bfloat16
    qT = np.ascontiguousarray(
        iQ.reshape(Bv * Qv, D).T.astype(bf))
    kT = np.ascontiguousarray(iK.reshape(Bv * Sv, D).T.astype(bf))
    keepT = np.ascontiguousarray(
        (~mask).transpose(0, 2, 1).astype(bf))
    scale = 1.0 / np.sqrt(ADIM)
    bpat_np = np.zeros((2, P), bf)
    bpat_np[0, 0:ADIM] = 1.0
    bpat_np[1, ADIM:2 * ADIM] = 1.0
    in_maps = []
    for c in range(NCORES):
        hsl = slice(P * c, P * (c + 1))
        in_maps.append({
            "qT": qT,
            "kT": kT,
            "keepT": keepT,
            "wqT": np.ascontiguousarray((Wq[hsl, :] * scale).T.astype(bf)),
            "wkT": np.ascontiguousarray(Wkv[hsl, :].T.astype(bf)),
            "wvT": np.ascontiguousarray(
                Wkv[D + P * c : D + P * (c + 1), :].T.astype(bf)),
            "woT": np.ascontiguousarray(Wo[:, hsl].T.astype(bf)),
            "bpat": bpat_np,
        })
    return in_maps


def kernel(iQ, iK, mask, Wq, Wkv, Wo, nbias):
    global _last_results
    iQ = np.asarray(iQ, np.float32)
    iK = np.asarray(iK, np.float32)
    mask = np.asarray(mask)
    Wq = np.asarray(Wq, np.float32)
    Wkv = np.asarray(Wkv, np.float32)
    Wo = np.asarray(Wo, np.float32)
    nbias = np.asarray(nbias, np.float32)
    Bv, Qv, _ = iQ.shape
    Sv = iK.shape[1]

    nc = _build(Bv, Qv, Sv, float(nbias[0]))
    in_maps = _prep_inputs(iQ, iK, mask, Wq, Wkv, Wo, nbias)
    trace = bool(int(os.environ.get("KERNEL_TRACE", "0")))
    res = run_bass_kernel_spmd(
        nc, in_maps, core_ids=list(range(NCORES)), trace=trace)
    _last_results = res
    total = np.zeros((Bv * Qv, D), np.float32)
    for r in res.results:
        total += r["out"]
    return total.reshape(Bv, Qv, D)


# revision 9
# speedup vs baseline: 30703.2774x; 30703.2774x over previous
"""CrossAttn + SparseNormer TRN2 kernel, tensor-parallel over heads on 8 cores.

Sharding: core c computes heads {2c, 2c+1} end-to-end (Wq/Wkv sharded on
output dim, Wo on input dim); each core emits a partial (B*Q, D) output of
the final projection and the host sums the 8 partials (the "all-reduce").

Per-core dataflow (all layouts chosen so no on-device transposes are needed):
  rqT[c, bq] = (Wq_c/sqrt(ADIM)) @ iQ.T      (fp32 in SBUF, f32r matmuls)
  rkT[c, bs] = Wk_c @ iK.T
  rvo[s, (b,sc), 0:64|64] = iK @ Wv_c.T with a constant 1.0 column appended
  scoresT[s, q] = rkT.T @ rqT per head       (K=64, row-packed 2 heads)
  t = relu(scoresT + nbias)^2 * keep  ==  (r*keep)*r  with r = relu(s+b)
  po[0:64|64, q] = [rv | 1].T @ t            (attn-value + rowsum in one MM)
  oT = po[0:64] / (po[64] + eps)             (recip + PE-broadcast + mult)
  out_partial = oT.T @ Wo_c.T                (K=64 x2 heads accumulated)
"""

import os
import numpy as np
import ml_dtypes
from contextlib import ExitStack

import concourse.bass as bass
import concourse.tile as tile
from concourse import bacc, mybir
from concourse.bass import ts, ds
from concourse.bass_utils import run_bass_kernel_spmd

AF = mybir.ActivationFunctionType
ALU = mybir.AluOpType
F32 = mybir.dt.float32
F32R = mybir.dt.float32r
BF16 = mybir.dt.bfloat16

B, Q, S, D, H = 2, 2048, 2048, 1024, 16
ADIM = 64
NCORES = 8
P = 128
QB = 512          # q-block (psum free dim)
SCH = 128         # s-chunk (scores partition dim)
IEPS = 1e-32

GPS_EVERY = 2     # route every GPS_EVERY-th square-mult to gpsimd (0=off)

_last_results = None


def _body(ctx, tc, aps, Bv, Qv, Sv, nbias_val):
    nc = tc.nc
    qT, kT, keepT, wqT, wkT, wvT, woT, bpat, out = aps
    BQ, BS = Bv * Qv, Bv * Sv
    KC = D // P                      # contraction chunks for projections
    nsc_b = Sv // SCH                # s-chunks per batch
    nqt_b = Qv // QB                 # q-blocks per batch
    nsb_tot = BS // SCH              # total s-chunks

    qT_r = qT.rearrange("(o p) n -> p o n", p=P)
    kT_r = kT.rearrange("(o p) n -> p o n", p=P)

    const = ctx.enter_context(tc.tile_pool(name="const", bufs=1))
    wq_sb = const.tile([P, KC, P], BF16)
    nc.sync.dma_start(wq_sb, wqT.rearrange("(o p) m -> p o m", p=P))
    wk_sb = const.tile([P, KC, P], BF16)
    nc.sync.dma_start(wk_sb, wkT.rearrange("(o p) m -> p o m", p=P))
    wv_sb = const.tile([P, KC, P], BF16)
    nc.sync.dma_start(wv_sb, wvT.rearrange("(o p) m -> p o m", p=P))
    # Wo split per head so lhsT/rhs partition bases line up at 0
    woA_sb = const.tile([ADIM, D], BF16)
    nc.sync.dma_start(woA_sb, woT[0:ADIM, :])
    woB_sb = const.tile([ADIM, D], BF16)
    nc.sync.dma_start(woB_sb, woT[ADIM : 2 * ADIM, :])
    # broadcast pattern (host const): cols 0:64 = [[1],[0]], 64:128 = [[0],[1]]
    bpat_sb = const.tile([2, P], BF16)
    nc.sync.dma_start(bpat_sb, bpat)
    bpatA = bpat_sb[:, 0:ADIM]
    bpatB = bpat_sb[:, ADIM : 2 * ADIM]

    rqT = const.tile([P, BQ], F32R)
    rkT = const.tile([P, BS], F32R)
    rvoA = const.tile([P, nsb_tot, ADIM + 1], BF16)
    nc.any.memset(rvoA[:, :, ADIM], 1.0)
    rvoB = const.tile([P, nsb_tot, ADIM + 1], BF16)
    nc.any.memset(rvoB[:, :, ADIM], 1.0)

    io = ctx.enter_context(tc.tile_pool(name="io", bufs=3))

    # ---------------- phase 1: projections ----------------
    with tc.tile_pool(name="pp", bufs=2, space="PSUM") as pp:
        for t in range(BQ // QB):
            qld = io.tile([P, KC, QB], BF16, tag="qload")
            nc.sync.dma_start(qld, qT_r[:, :, ts(t, QB)])
            ps = pp.tile([P, QB], F32, tag="pp")
            for kc in range(KC):
                nc.tensor.matmul(ps, wq_sb[:, kc, :], qld[:, kc, :],
                                 start=(kc == 0), stop=(kc == KC - 1))
            nc.scalar.copy(rqT[:, ts(t, QB)], ps)

        for t in range(BS // QB):
            kld = io.tile([P, KC, QB], BF16, tag="qload")
            nc.sync.dma_start(kld, kT_r[:, :, ts(t, QB)])
            ps = pp.tile([P, QB], F32, tag="pp")
            for kc in range(KC):
                nc.tensor.matmul(ps, wk_sb[:, kc, :], kld[:, kc, :],
                                 start=(kc == 0), stop=(kc == KC - 1))
            nc.scalar.copy(rkT[:, ts(t, QB)], ps)
            for j in range(QB // SCH):
                sidx = t * (QB // SCH) + j
                prv = pp.tile([P, P], F32, tag="prv")
                for kc in range(KC):
                    nc.tensor.matmul(prv, kld[:, kc, ds(j * SCH, SCH)],
                                     wv_sb[:, kc, :],
                                     start=(kc == 0), stop=(kc == KC - 1))
                nc.vector.tensor_copy(rvoA[:, sidx, 0:ADIM], prv[:, 0:ADIM])
                nc.vector.tensor_copy(rvoB[:, sidx, 0:ADIM],
                                      prv[:, ADIM : 2 * ADIM])

    # ---------------- phase 2: attention ----------------
    sp = ctx.enter_context(tc.tile_pool(name="sp", bufs=4, space="PSUM"))
    pop = ctx.enter_context(tc.tile_pool(name="pop", bufs=1, space="PSUM"))
    op = ctx.enter_context(tc.tile_pool(name="op", bufs=2, space="PSUM"))
    bcp = op  # bcast + outproj share the same 2 banks (short-lived tiles)
    sb2 = ctx.enter_context(tc.tile_pool(name="sb2", bufs=5))
    sb3 = ctx.enter_context(tc.tile_pool(name="sb3", bufs=2))
    kp = ctx.enter_context(tc.tile_pool(name="kp", bufs=3))

    gps_ctr = 0
    for b in range(Bv):
        for qt in range(nqt_b):
            qs = b * Qv + qt * QB
            poA = pop.tile([ADIM + 1, QB], F32, tag="poA")
            poB = pop.tile([ADIM + 1, QB], F32, tag="poB")
            # software-pipelined: attnV for s-chunk sc-1 is emitted after the
            # scores matmuls for sc, so PE never stalls on the ACT/DVE chain
            pend = None  # (sidx, ttA, ttB)
            for sc in range(nsc_b):
                ss = b * Sv + sc * SCH
                sidx = b * nsc_b + sc
                keep_t = kp.tile([SCH, QB], BF16, tag="keep")
                nc.sync.dma_start(
                    keep_t, keepT[b, ds(sc * SCH, SCH), ds(qt * QB, QB)])
                tts = []
                for half in (0, 1):
                    hs = half * ADIM
                    pscore = sp.tile([SCH, QB], F32, tag="ps_sc")
                    nc.tensor.matmul(
                        pscore,
                        rkT[hs : hs + ADIM, ds(ss, SCH)],
                        rqT[hs : hs + ADIM, ds(qs, QB)],
                        start=True, stop=True, tile_position=(hs, 0))
                    r = sb2.tile([SCH, QB], BF16, tag="r")
                    nc.scalar.activation(r, pscore, AF.Relu,
                                         bias=float(nbias_val), scale=1.0)
                    rk_ = sb2.tile([SCH, QB], BF16, tag="rk")
                    nc.vector.tensor_tensor(rk_, r, keep_t, op=ALU.mult)
                    tt = sb2.tile([SCH, QB], BF16, tag="tt")
                    gps_ctr += 1
                    if GPS_EVERY and gps_ctr % GPS_EVERY == 0:
                        nc.gpsimd.tensor_tensor(tt, rk_, r, op=ALU.mult)
                    else:
                        nc.vector.tensor_tensor(tt, rk_, r, op=ALU.mult)
                    tts.append(tt)
                if pend is not None:
                    psidx, pA, pB = pend
                    nc.tensor.matmul(poA, rvoA[:, psidx, :], pA,
                                     start=(psidx % nsc_b == 0), stop=False)
                    nc.tensor.matmul(poB, rvoB[:, psidx, :], pB,
                                     start=(psidx % nsc_b == 0), stop=False)
                pend = (sidx, tts[0], tts[1])
            psidx, pA, pB = pend
            nc.tensor.matmul(poA, rvoA[:, psidx, :], pA,
                             start=(nsc_b == 1), stop=True)
            nc.tensor.matmul(poB, rvoB[:, psidx, :], pB,
                             start=(nsc_b == 1), stop=True)

            # normalization
            poA_sb = sb3.tile([ADIM + 1, QB], F32, tag="poA_sb")
            nc.scalar.copy(poA_sb, poA)
            poB_sb = sb3.tile([ADIM + 1, QB], F32, tag="poB_sb")
            nc.scalar.copy(poB_sb, poB)
            rs2 = sb3.tile([2, QB], F32, tag="rs2")
            nc.sync.dma_start(rs2[0:1, :], poA_sb[ADIM : ADIM + 1, :])
            nc.sync.dma_start(rs2[1:2, :], poB_sb[ADIM : ADIM + 1, :])
            rcp = sb3.tile([2, QB], F32, tag="rcp")
            nc.vector.tensor_scalar_add(rcp, rs2, IEPS)
            nc.vector.reciprocal(rcp, rcp)
            rcpb = sb3.tile([2, QB], BF16, tag="rcpb")
            nc.vector.tensor_copy(rcpb, rcp)
            bcA = bcp.tile([ADIM, QB], F32, tag="px")
            nc.tensor.matmul(bcA, bpatA, rcpb, start=True, stop=True)
            bcB = bcp.tile([ADIM, QB], F32, tag="px")
            nc.tensor.matmul(bcB, bpatB, rcpb, start=True, stop=True)
            oTA = sb3.tile([ADIM, QB], BF16, tag="oTA")
            nc.vector.tensor_tensor(oTA, poA_sb[0:ADIM, :], bcA, op=ALU.mult)
            oTB = sb3.tile([ADIM, QB], BF16, tag="oTB")
            nc.vector.tensor_tensor(oTB, poB_sb[0:ADIM, :], bcB, op=ALU.mult)

            # output projection (partial over this core's 128 f-dims)
            for qc in range(QB // P):
                for ec in range(D // QB):
                    pso = op.tile([P, QB], F32, tag="px")
                    nc.tensor.matmul(pso, oTA[:, ds(qc * P, P)],
                                     woA_sb[:, ds(ec * QB, QB)],
                                     start=True, stop=False)
                    nc.tensor.matmul(pso, oTB[:, ds(qc * P, P)],
                                     woB_sb[:, ds(ec * QB, QB)],
                                     start=False, stop=True)
                    osb = sb2.tile([P, QB], F32, tag="osb")
                    nc.any.tensor_copy(osb, pso)
                    nc.sync.dma_start(
                        out[ds(qs + qc * P, P), ds(ec * QB, QB)], osb)


_nc_cache = {}


def _build(Bv, Qv, Sv, nbias_val, num_devices=NCORES):
    key = (Bv, Qv, Sv, float(nbias_val), num_devices)
    if key in _nc_cache:
        return _nc_cache[key]
    nc = bacc.Bacc("TRN2", target_bir_lowering=False, debug=False,
                   num_devices=num_devices)
    BQ, BS = Bv * Qv, Bv * Sv
    qT = nc.dram_tensor("qT", [D, BQ], BF16, kind="ExternalInput").ap()
    kT = nc.dram_tensor("kT", [D, BS], BF16, kind="ExternalInput").ap()
    keepT = nc.dram_tensor("keepT", [Bv, Sv, Qv], BF16,
                           kind="ExternalInput").ap()
    wqT = nc.dram_tensor("wqT", [D, P], BF16, kind="ExternalInput").ap()
    wkT = nc.dram_tensor("wkT", [D, P], BF16, kind="ExternalInput").ap()
    wvT = nc.dram_tensor("wvT", [D, P], BF16, kind="ExternalInput").ap()
    woT = nc.dram_tensor("woT", [P, D], BF16, kind="ExternalInput").ap()
    bpat = nc.dram_tensor("bpat", [2, P], BF16, kind="ExternalInput").ap()
    out = nc.dram_tensor("out", [BQ, D], F32, kind="ExternalOutput").ap()
    aps = (qT, kT, keepT, wqT, wkT, wvT, woT, bpat, out)
    with tile.TileContext(nc) as tc:
        with ExitStack() as ctx:
            _body(ctx, tc, aps, Bv, Qv, Sv, nbias_val)
    nc.compile()
    _nc_cache[key] = nc
    return nc


def _prep_inputs(iQ, iK, mask, Wq, Wkv, Wo, nbias):
    Bv, Qv, _ = iQ.shape
    Sv = iK.shape[1]
    bf = ml_dtypes.bfloat16
    qT = np.ascontiguousarray(
        iQ.reshape(Bv * Qv, D).T.astype(bf))
    kT = np.ascontiguousarray(iK.reshape(Bv * Sv, D).T.astype(bf))
    keepT = np.ascontiguousarray(
        (~mask).transpose(0, 2, 1).astype(bf))
    scale = 1.0 / np.sqrt(ADIM)
    bpat_np = np.zeros((2, P), bf)
    bpat_np[0, 0:ADIM] = 1.0
    bpat_np[1, ADIM:2 * ADIM] = 1.0
    in_maps = []
    for c in range(NCORES):
        hsl = slice(P * c, P * (c + 1))
        in_maps.append({
            "qT": qT,
            "kT": kT,
            "keepT": keepT,
            "wqT": np.ascontiguousarray((Wq[hsl, :] * scale).T.astype(bf)),
            "wkT": np.ascontiguousarray(Wkv[hsl, :].T.astype(bf)),
            "wvT": np.ascontiguousarray(
                Wkv[D + P * c : D + P * (c + 1), :].T.astype(bf)),
            "woT": np.ascontiguousarray(Wo[:, hsl].T.astype(bf)),
            "bpat": bpat_np,
        })
    return in_maps


def kernel(iQ, iK, mask, Wq, Wkv, Wo, nbias):
    global _last_results
    iQ = np.asarray(iQ, np.float32)
    iK = np.asarray(iK, np.float32)
    mask = np.asarray(mask)
    Wq = np.asarray(Wq, np.float32)
    Wkv = np.asarray(Wkv, np.float32)
    Wo = np.asarray(Wo, np.float32)
    nbias = np.asarray(nbias, np.float32)
    Bv, Qv, _ = iQ.shape
    Sv = iK.shape[1]

    nc = _build(Bv, Qv, Sv, float(nbias[0]))
    in_maps = _prep_inputs(iQ, iK, mask, Wq, Wkv, Wo, nbias)
    trace = bool(int(os.environ.get("KERNEL_TRACE", "0")))
    res = run_bass_kernel_spmd(
        nc, in_maps, core_ids=list(range(NCORES)), trace=trace)
    _last_results = res
    total = np.zeros((Bv * Qv, D), np.float32)
    for r in res.results:
        total += r["out"]
    return total.reshape(Bv, Qv, D)
